# revision 16
# baseline (speedup 1.0000x reference)
"""GCN layer (per-node linear + sparse aggregation) on 8 Trainium2 cores.

Strategy (sharding_hint: shard nodes across devices):
  - Nodes sharded 1250/core (padded to 1280 = 10 blocks of 128).
  - Phase A (per core): h[b,n,:] = x[b,n,:] @ kernel[n]   for local nodes.
    kernel/x pre-cast to bf16 and laid out f-major on host so every DMA is
    contiguous per partition. Per node: matmul(lhsT=K[n] [f,g], rhs=xT[n]
    [f,b]) -> psum [g,b]; per 128-node block transpose to H rows
    [n, b*128+g] (bf16) and store.
  - Host: concat per-core H shards -> full H [10240, 512] (slot space).
  - Phase B (per core): edges partitioned by src slot (nodes assigned to
    cores/blocks by degree-balanced LPT so all blocks have ~equal edge
    counts), grouped per 128-src block into chunks of 128 edges, sorted by
    dst. Per block: dma_gather of H rows for edge dsts (<=1024 idxs per
    call) -> msgs; S[e, src_off] = w built on DVE via is_equal/mult against
    a column-index constant; segment-sum via PE: psum[128 src, 512] +=
    S_chunk.T @ msgs_chunk; bias added by DVE; bf16 out rows.
  - Host: un-permute rows -> out [4, 10000, 128] f32.
"""

import numpy as np
import ml_dtypes

import concourse.bass as bass
import concourse.bacc as bacc
import concourse.mybir as mybir
import concourse.tile as tile
from concourse import library_config
from concourse.bass_utils import run_bass_kernel_spmd

BF16 = ml_dtypes.bfloat16

B, N, F, E = 4, 10000, 128, 160000
NC = 8
NSH = N // NC          # 1250 nodes per core
NPAD = 1280            # padded to 10 blocks of 128
BLK = 128
NBLK = NPAD // BLK     # 10
ROW = B * F            # 512 = H row width

_CACHE: dict = {}


def _build_phase_a():
    if "a" in _CACHE:
        return _CACHE["a"]
    nc = bacc.Bacc("TRN2", target_bir_lowering=False, debug=False)
    kt = nc.dram_tensor("kt", [F, NPAD * F], mybir.dt.bfloat16, kind="ExternalInput").ap()
    xt = nc.dram_tensor("xt", [F, NPAD * B], mybir.dt.bfloat16, kind="ExternalInput").ap()
    ident = nc.dram_tensor("ident", [128, 128], mybir.dt.bfloat16, kind="ExternalInput").ap()
    hout = nc.dram_tensor("hout", [NPAD, B, F], mybir.dt.bfloat16, kind="ExternalOutput").ap()

    with tile.TileContext(nc) as tc:
        with (
            tc.tile_pool(name="kpool", bufs=6) as kpool,
            tc.tile_pool(name="xpool", bufs=1) as xpool,
            tc.tile_pool(name="ipool", bufs=1) as ipool,
            tc.tile_pool(name="tpool", bufs=2) as tpool,
            tc.tile_pool(name="hpool", bufs=2) as hpool,
            tc.tile_pool(name="psA", bufs=2, space="PSUM") as psA,
            tc.tile_pool(name="psT", bufs=2, space="PSUM") as psT,
        ):
            xtile = xpool.tile([128, NPAD * B], mybir.dt.bfloat16)
            nc.scalar.dma_start(xtile[:], xt[:])
            idt = ipool.tile([128, 128], mybir.dt.bfloat16)
            nc.scalar.dma_start(idt[:], ident[:])

            for blk in range(NBLK):
                kts = []
                for h in range(2):
                    kth = kpool.tile([128, BLK * F // 2], mybir.dt.bfloat16, tag="kt")
                    nc.sync.dma_start(
                        kth[:], kt[:, (2 * blk + h) * (BLK // 2) * F:(2 * blk + h + 1) * (BLK // 2) * F])
                    kts.append(kth)

                pa = psA.tile([128, BLK, B], mybir.dt.float32)
                for j in range(BLK):
                    nc.tensor.matmul(
                        pa[:, j, :],
                        kts[j // 64][:, (j % 64) * F:(j % 64 + 1) * F],
                        xtile[:, (blk * BLK + j) * B:(blk * BLK + j + 1) * B],
                        start=True, stop=True,
                    )
                # pa is [g, n, b]; cast+split per b, transpose to [n, b, g]
                sb = tpool.tile([128, B, 128], mybir.dt.bfloat16)
                for b in range(B):
                    nc.vector.tensor_copy(sb[:, b, :], pa[:, :, b])
                pt = psT.tile([128, B, 128], mybir.dt.bfloat16)
                for b in range(B):
                    nc.tensor.transpose(pt[:, b, :], sb[:, b, :], idt[:])
                hs = hpool.tile([128, B, 128], mybir.dt.bfloat16)
                nc.vector.tensor_copy(hs[:], pt[:])
                nc.scalar.dma_start(hout[blk * BLK:(blk + 1) * BLK], hs[:])

    nc.compile()
    _CACHE["a"] = nc
    return nc


def _build_phase_b(nchunk: int):
    key = ("b", nchunk)
    if key in _CACHE:
        return _CACHE[key]
    L = nchunk * 128
    GSZ = 8                       # gather group: 8 chunks = 1024 idxs (ring limit)
    nc = bacc.Bacc("TRN2", target_bir_lowering=False, debug=False)
    hx = nc.dram_tensor("hx", [NC * NPAD, ROW], mybir.dt.bfloat16, kind="ExternalInput").ap()
    brow = nc.dram_tensor("brow", [128, NBLK * ROW], mybir.dt.bfloat16, kind="ExternalInput").ap()
    ix = nc.dram_tensor("ix", [128, NBLK * (L // 16)], mybir.dt.int16, kind="ExternalInput").ap()
    ow = nc.dram_tensor("ow", [128, NBLK * 2 * nchunk], mybir.dt.float32, kind="ExternalInput").ap()
    colx = nc.dram_tensor("colx", [128, 128], mybir.dt.bfloat16, kind="ExternalInput").ap()
    oout = nc.dram_tensor("oout", [NPAD, ROW], mybir.dt.bfloat16, kind="ExternalOutput").ap()

    ngrp = (nchunk + GSZ - 1) // GSZ

    with tile.TileContext(nc) as tc:
        nc.gpsimd.load_library(library_config.mlp)
        with (
            tc.tile_pool(name="cpool", bufs=1) as cpool,
            tc.tile_pool(name="mpool", bufs=3 * ngrp) as mpool,
            tc.tile_pool(name="spool", bufs=3) as spool,
            tc.tile_pool(name="opool", bufs=2) as opool,
            tc.tile_pool(name="pspool", bufs=3, space="PSUM") as pspool,
        ):
            ct = cpool.tile([128, 128], mybir.dt.bfloat16)
            nc.scalar.dma_start(ct[:], colx[:])
            ixt = cpool.tile([128, NBLK * (L // 16)], mybir.dt.int16, tag="ixall")
            nc.scalar.dma_start(ixt[:], ix[:])
            owt = cpool.tile([128, NBLK * 2 * nchunk], mybir.dt.float32, tag="owall")
            nc.scalar.dma_start(owt[:], ow[:])
            bts = cpool.tile([128, NBLK * ROW], mybir.dt.bfloat16, tag="ball")
            nc.scalar.dma_start(bts[:], brow[:])

            for blk in range(NBLK):
                ixb = ixt[:, blk * (L // 16):(blk + 1) * (L // 16)]
                oft = owt[:, blk * 2 * nchunk:blk * 2 * nchunk + nchunk]
                wst = owt[:, blk * 2 * nchunk + nchunk:(blk + 1) * 2 * nchunk]
                bt = bts[:, blk * ROW:(blk + 1) * ROW]

                # gather H rows for this block's edges, 1024 idxs per call
                mts = []
                for g in range(ngrp):
                    gc = min(GSZ, nchunk - g * GSZ)
                    mt = mpool.tile([128, GSZ, ROW], mybir.dt.bfloat16, tag="msgs")
                    nc.gpsimd.dma_gather(
                        mt[:, :gc, :], hx[:],
                        ixb[:, g * GSZ * 8:(g * GSZ + gc) * 8],
                        gc * 128, gc * 128, ROW,
                    )
                    mts.append(mt)

                # build S on DVE: S[e, m] = (m == off[e]) * w[e]
                st = spool.tile([128, L], mybir.dt.bfloat16)
                for c in range(nchunk):
                    nc.vector.tensor_scalar(
                        st[:, c * 128:(c + 1) * 128], ct[:],
                        oft[:, c:c + 1], wst[:, c:c + 1],
                        mybir.AluOpType.is_equal, mybir.AluOpType.mult,
                    )

                ps = pspool.tile([128, ROW], mybir.dt.float32)
                for c in range(nchunk):
                    nc.tensor.matmul(
                        ps[:],
                        st[:, c * 128:(c + 1) * 128],
                        mts[c // GSZ][:, c % GSZ, :],
                        start=(c == 0), stop=(c == nchunk - 1),
                    )
                ot = opool.tile([128, ROW], mybir.dt.bfloat16)
                nc.vector.tensor_add(ot[:], ps[:], bt[:])
                nc.sync.dma_start(oout[blk * BLK:(blk + 1) * BLK, :], ot[:])

    nc.compile()
    _CACHE[key] = nc
    return nc


def kernel(x, kernel, bias, edge_src, edge_dst, edge_w):
    x = np.asarray(x, dtype=np.float32)
    kernel = np.asarray(kernel, dtype=np.float32)
    bias = np.asarray(bias, dtype=np.float32)
    edge_src = np.asarray(edge_src, dtype=np.int32)
    edge_dst = np.asarray(edge_dst, dtype=np.int32)
    edge_w = np.asarray(edge_w, dtype=np.float32)

    ident = np.eye(128, dtype=BF16)

    # ---- host: degree-balanced node -> (core, slot) assignment (LPT) ----
    # Blocks of 128 slots; balance per-block edge counts so the worst block
    # (which sets nchunk for everyone) is near the mean.
    import heapq
    deg = np.bincount(edge_src, minlength=N)
    NBINS = NC * NBLK                     # 80 blocks
    cap = [125 + (1 if i < (N - 125 * NBINS) else 0) for i in range(NBINS)]
    heap = [(0, i) for i in range(NBINS)]
    heapq.heapify(heap)
    fill = [0] * NBINS
    bin_nodes: list[list[int]] = [[] for _ in range(NBINS)]
    for node in np.argsort(-deg, kind="stable"):
        spill = []
        while True:
            d, i = heapq.heappop(heap)
            if fill[i] < cap[i]:
                break
            spill.append((d, i))
        for s in spill:
            heapq.heappush(heap, s)
        bin_nodes[i].append(int(node))
        fill[i] += 1
        heapq.heappush(heap, (d + int(deg[node]), i))

    node_of_slot = np.full((NC, NPAD), -1, dtype=np.int64)
    for b in range(NBINS):
        c, blk = divmod(b, NBLK)
        nodes = bin_nodes[b]
        node_of_slot[c, blk * BLK:blk * BLK + len(nodes)] = nodes
    slot_core = np.empty(N, dtype=np.int64)
    slot_idx = np.empty(N, dtype=np.int64)
    for c in range(NC):
        real = node_of_slot[c] >= 0
        slot_core[node_of_slot[c][real]] = c
        slot_idx[node_of_slot[c][real]] = np.nonzero(real)[0]

    # ---- host prep: phase A inputs (permuted shards) ----
    import time
    _tm = {}
    _t = time.time()
    in_a = []
    for c in range(NC):
        nos = node_of_slot[c]
        real = nos >= 0
        kc = np.zeros((NPAD, F, F), dtype=BF16)
        kc[real] = kernel[nos[real]].astype(BF16)
        kt = np.ascontiguousarray(kc.transpose(1, 0, 2)).reshape(F, NPAD * F)
        xc = np.zeros((B, NPAD, F), dtype=BF16)
        xc[:, real] = x[:, nos[real]].astype(BF16)
        xt = np.ascontiguousarray(xc.transpose(2, 1, 0)).reshape(F, NPAD * B)
        in_a.append({"kt": kt, "xt": xt, "ident": ident})

    _tm["prep_a"] = time.time() - _t
    nc_a = _build_phase_a()
    _t = time.time()
    res_a = run_bass_kernel_spmd(nc_a, in_a, core_ids=list(range(NC)))
    _tm["run_a"] = time.time() - _t

    # full H in slot space: row (c*NPAD + slot) = h of node_of_slot[c, slot]
    Hfull = np.concatenate(
        [res_a.results[c]["hout"].reshape(NPAD, ROW) for c in range(NC)], axis=0)

    # ---- host: edge partition by src slot ----
    e_core = slot_core[edge_src]
    e_slot = slot_idx[edge_src]
    e_dstrow = (slot_core[edge_dst] * NPAD + slot_idx[edge_dst]).astype(np.int16)

    counts = np.zeros((NC, NBLK), dtype=np.int64)
    groups: list[list] = [[None] * NBLK for _ in range(NC)]
    for c in range(NC):
        m = e_core == c
        lc, dc, wc = e_slot[m], e_dstrow[m], edge_w[m]
        for blk in range(NBLK):
            mb = (lc // BLK) == blk
            # sort by dst row: ascending-address gather descriptors
            o2 = np.argsort(dc[mb], kind="stable")
            groups[c][blk] = ((lc[mb] % BLK)[o2], dc[mb][o2], wc[mb][o2])
            counts[c, blk] = int(mb.sum())

    nchunk = int(np.ceil(counts.max() / 128))
    nchunk += nchunk % 2          # quantize to even for compile-cache stability
    L = nchunk * 128

    colx = np.tile(np.arange(128, dtype=BF16), (128, 1))
    _t = time.time()
    in_b = []
    for c in range(NC):
        nos = node_of_slot[c]
        real = nos >= 0
        bc = np.zeros((NPAD, F), dtype=np.float32)
        bc[real] = bias[nos[real]]
        brow = np.tile(bc, (1, B)).reshape(NBLK, 128, ROW).astype(BF16)  # [n, b*128+g] rows

        owh = np.zeros((NBLK, 128, 2 * nchunk), dtype=np.float32)
        ixh = np.zeros((NBLK, 16, L // 16), dtype=np.int16)
        for blk in range(NBLK):
            off, dst, w = groups[c][blk]
            n = len(off)
            i = np.arange(n)
            owh[blk, i % 128, i // 128] = off
            owh[blk, i % 128, nchunk + i // 128] = w.astype(BF16)
            ixh[blk, i % 16, i // 16] = dst
        ixh = np.tile(ixh, (1, 8, 1))                   # replicate to 128 partitions
        # partition-contiguous layouts: [128, NBLK*...]
        brow2 = np.ascontiguousarray(brow.transpose(1, 0, 2)).reshape(128, NBLK * ROW)
        ix2 = np.ascontiguousarray(ixh.transpose(1, 0, 2)).reshape(128, NBLK * (L // 16))
        ow2 = np.ascontiguousarray(owh.transpose(1, 0, 2)).reshape(128, NBLK * 2 * nchunk)
        in_b.append({"hx": Hfull, "brow": brow2, "ix": ix2,
                     "ow": ow2, "colx": colx})

    _tm["prep_b"] = time.time() - _t
    nc_b = _build_phase_b(nchunk)
    _t = time.time()
    res_b = run_bass_kernel_spmd(nc_b, in_b, core_ids=list(range(NC)))
    _tm["run_b"] = time.time() - _t

    _t = time.time()
    out = np.empty((B, N, F), dtype=np.float32)
    for c in range(NC):
        nos = node_of_slot[c]
        real = nos >= 0
        o = res_b.results[c]["oout"][real].astype(np.float32).reshape(-1, B, F).transpose(1, 0, 2)
        out[:, nos[real]] = o
    _tm["assemble"] = time.time() - _t
    import sys as _sys
    print("kernel timing:", {k: round(v, 2) for k, v in _tm.items()}, file=_sys.stderr)
    return out


# revision 20
# speedup vs baseline: 1.0133x; 1.0133x over previous
"""GCN layer (per-node linear + sparse aggregation) on 8 Trainium2 cores.

Strategy (sharding_hint: shard nodes across devices):
  - Nodes sharded 1250/core (padded to 1280 = 10 blocks of 128).
  - Phase A (per core): h[b,n,:] = x[b,n,:] @ kernel[n]   for local nodes.
    kernel/x pre-cast to bf16 and laid out f-major on host so every DMA is
    contiguous per partition. Per node: matmul(lhsT=K[n] [f,g], rhs=xT[n]
    [f,b]) -> psum [g,b]; per 128-node block transpose to H rows
    [n, b*128+g] (bf16) and store.
  - Host: concat per-core H shards -> full H [10240, 512] (slot space).
  - Phase B (per core): edges partitioned by src slot (nodes assigned to
    cores/blocks by degree-balanced LPT so all blocks have ~equal edge
    counts), grouped per 128-src block into chunks of 128 edges, sorted by
    dst. Per block: dma_gather of H rows for edge dsts (<=1024 idxs per
    call) -> msgs; S[e, src_off] = w built on DVE via is_equal/mult against
    a column-index constant; segment-sum via PE: psum[128 src, 512] +=
    S_chunk.T @ msgs_chunk; bias added by DVE; bf16 out rows.
  - Host: un-permute rows -> out [4, 10000, 128] f32.
"""

import numpy as np
import ml_dtypes

import concourse.bass as bass
import concourse.bacc as bacc
import concourse.mybir as mybir
import concourse.tile as tile
from concourse import library_config
from concourse.bass_utils import run_bass_kernel_spmd

BF16 = ml_dtypes.bfloat16

B, N, F, E = 4, 10000, 128, 160000
NC = 8
NSH = N // NC          # 1250 nodes per core
NPAD = 1280            # padded to 10 blocks of 128
BLK = 128
NBLK = NPAD // BLK     # 10
ROW = B * F            # 512 = H row width

_CACHE: dict = {}


def _build_phase_a():
    if "a" in _CACHE:
        return _CACHE["a"]
    nc = bacc.Bacc("TRN2", target_bir_lowering=False, debug=False)
    kt = nc.dram_tensor("kt", [F, NBLK * 125 * F], mybir.dt.bfloat16, kind="ExternalInput").ap()
    xt = nc.dram_tensor("xt", [F, NPAD * B], mybir.dt.bfloat16, kind="ExternalInput").ap()
    ident = nc.dram_tensor("ident", [128, 128], mybir.dt.bfloat16, kind="ExternalInput").ap()
    hout = nc.dram_tensor("hout", [NPAD, B, F], mybir.dt.bfloat16, kind="ExternalOutput").ap()

    with tile.TileContext(nc) as tc:
        with (
            tc.tile_pool(name="kpool", bufs=6) as kpool,
            tc.tile_pool(name="xpool", bufs=1) as xpool,
            tc.tile_pool(name="ipool", bufs=1) as ipool,
            tc.tile_pool(name="tpool", bufs=2) as tpool,
            tc.tile_pool(name="hpool", bufs=2) as hpool,
            tc.tile_pool(name="psA", bufs=2, space="PSUM") as psA,
            tc.tile_pool(name="psT", bufs=2, space="PSUM") as psT,
        ):
            xtile = xpool.tile([128, NPAD * B], mybir.dt.bfloat16)
            nc.scalar.dma_start(xtile[:], xt[:])
            idt = ipool.tile([128, 128], mybir.dt.bfloat16)
            nc.scalar.dma_start(idt[:], ident[:])

            for blk in range(NBLK):
                # 125 real nodes per block, split 63 + 62 for pipelining
                kts = []
                for h, (c0, cn) in enumerate(((0, 63), (63, 62))):
                    kth = kpool.tile([128, 63 * F], mybir.dt.bfloat16, tag="kt")
                    nc.sync.dma_start(
                        kth[:, :cn * F],
                        kt[:, (blk * 125 + c0) * F:(blk * 125 + c0 + cn) * F])
                    kts.append(kth)

                pa = psA.tile([128, BLK, B], mybir.dt.float32)
                for j in range(125):
                    nc.tensor.matmul(
                        pa[:, j, :],
                        kts[j // 63][:, (j % 63) * F:(j % 63 + 1) * F],
                        xtile[:, (blk * BLK + j) * B:(blk * BLK + j + 1) * B],
                        start=True, stop=True,
                    )
                # pa is [g, n, b]; cast+split per b, transpose to [n, b, g]
                sb = tpool.tile([128, B, 128], mybir.dt.bfloat16)
                for b in range(B):
                    nc.vector.tensor_copy(sb[:, b, :], pa[:, :, b])
                pt = psT.tile([128, B, 128], mybir.dt.bfloat16)
                for b in range(B):
                    nc.tensor.transpose(pt[:, b, :], sb[:, b, :], idt[:])
                hs = hpool.tile([128, B, 128], mybir.dt.bfloat16)
                nc.vector.tensor_copy(hs[:], pt[:])
                nc.scalar.dma_start(hout[blk * BLK:(blk + 1) * BLK], hs[:])

    nc.compile()
    _CACHE["a"] = nc
    return nc


def _build_phase_b(nchunk: int):
    key = ("b", nchunk)
    if key in _CACHE:
        return _CACHE[key]
    L = nchunk * 128
    GSZ = 8                       # gather group: 8 chunks = 1024 idxs (ring limit)
    nc = bacc.Bacc("TRN2", target_bir_lowering=False, debug=False)
    hx = nc.dram_tensor("hx", [NC * NPAD, ROW], mybir.dt.bfloat16, kind="ExternalInput").ap()
    brow = nc.dram_tensor("brow", [128, NBLK * ROW], mybir.dt.bfloat16, kind="ExternalInput").ap()
    ix = nc.dram_tensor("ix", [128, NBLK * (L // 16)], mybir.dt.int16, kind="ExternalInput").ap()
    ow = nc.dram_tensor("ow", [128, NBLK * 2 * nchunk], mybir.dt.float32, kind="ExternalInput").ap()
    colx = nc.dram_tensor("colx", [128, 128], mybir.dt.bfloat16, kind="ExternalInput").ap()
    oout = nc.dram_tensor("oout", [NPAD, ROW], mybir.dt.bfloat16, kind="ExternalOutput").ap()

    ngrp = (nchunk + GSZ - 1) // GSZ

    with tile.TileContext(nc) as tc:
        nc.gpsimd.load_library(library_config.mlp)
        with (
            tc.tile_pool(name="cpool", bufs=1) as cpool,
            tc.tile_pool(name="mpool", bufs=3 * ngrp) as mpool,
            tc.tile_pool(name="spool", bufs=3) as spool,
            tc.tile_pool(name="opool", bufs=2) as opool,
            tc.tile_pool(name="pspool", bufs=2, space="PSUM") as pspool,
        ):
            ct = cpool.tile([128, 128], mybir.dt.bfloat16)
            nc.scalar.dma_start(ct[:], colx[:])
            ixt = cpool.tile([128, NBLK * (L // 16)], mybir.dt.int16, tag="ixall")
            nc.scalar.dma_start(ixt[:], ix[:])
            owt = cpool.tile([128, NBLK * 2 * nchunk], mybir.dt.float32, tag="owall")
            nc.scalar.dma_start(owt[:], ow[:])
            bts = cpool.tile([128, NBLK * ROW], mybir.dt.bfloat16, tag="ball")
            nc.scalar.dma_start(bts[:], brow[:])

            for blk in range(NBLK):
                ixb = ixt[:, blk * (L // 16):(blk + 1) * (L // 16)]
                oft = owt[:, blk * 2 * nchunk:blk * 2 * nchunk + nchunk]
                wst = owt[:, blk * 2 * nchunk + nchunk:(blk + 1) * 2 * nchunk]
                bt = bts[:, blk * ROW:(blk + 1) * ROW]

                # gather H rows for this block's edges, 1024 idxs per call
                mts = []
                for g in range(ngrp):
                    gc = min(GSZ, nchunk - g * GSZ)
                    mt = mpool.tile([128, GSZ, ROW], mybir.dt.bfloat16, tag="msgs")
                    nc.gpsimd.dma_gather(
                        mt[:, :gc, :], hx[:],
                        ixb[:, g * GSZ * 8:(g * GSZ + gc) * 8],
                        gc * 128, gc * 128, ROW,
                    )
                    mts.append(mt)

                # build S on DVE: S[e, m] = (m == off[e]) * w[e]
                st = spool.tile([128, L], mybir.dt.bfloat16)
                for c in range(nchunk):
                    nc.vector.tensor_scalar(
                        st[:, c * 128:(c + 1) * 128], ct[:],
                        oft[:, c:c + 1], wst[:, c:c + 1],
                        mybir.AluOpType.is_equal, mybir.AluOpType.mult,
                    )

                # one PSUM accumulation chain per gather group: each chain's
                # matmuls start as soon as its own gather lands
                pss = []
                for g in range(ngrp):
                    gc = min(GSZ, nchunk - g * GSZ)
                    ps = pspool.tile([128, ROW], mybir.dt.float32, tag=f"ps{g}")
                    for k in range(gc):
                        c = g * GSZ + k
                        nc.tensor.matmul(
                            ps[:],
                            st[:, c * 128:(c + 1) * 128],
                            mts[g][:, k, :],
                            start=(k == 0), stop=(k == gc - 1),
                        )
                    pss.append(ps)
                # DVE may read at most one PSUM operand per instruction
                ot = opool.tile([128, ROW], mybir.dt.bfloat16)
                if ngrp == 1:
                    nc.vector.tensor_add(ot[:], pss[0][:], bt[:])
                else:
                    acc = opool.tile([128, ROW], mybir.dt.float32, tag="acc")
                    nc.vector.tensor_add(acc[:], pss[0][:], bt[:])
                    for g in range(1, ngrp - 1):
                        nc.vector.tensor_add(acc[:], pss[g][:], acc[:])
                    nc.vector.tensor_add(ot[:], pss[ngrp - 1][:], acc[:])
                nc.sync.dma_start(oout[blk * BLK:(blk + 1) * BLK, :], ot[:])

    nc.compile()
    _CACHE[key] = nc
    return nc


def kernel(x, kernel, bias, edge_src, edge_dst, edge_w):
    x = np.asarray(x, dtype=np.float32)
    kernel = np.asarray(kernel, dtype=np.float32)
    bias = np.asarray(bias, dtype=np.float32)
    edge_src = np.asarray(edge_src, dtype=np.int32)
    edge_dst = np.asarray(edge_dst, dtype=np.int32)
    edge_w = np.asarray(edge_w, dtype=np.float32)

    ident = np.eye(128, dtype=BF16)

    # ---- host: degree-balanced node -> (core, slot) assignment (LPT) ----
    # Blocks of 128 slots; balance per-block edge counts so the worst block
    # (which sets nchunk for everyone) is near the mean.
    import heapq
    deg = np.bincount(edge_src, minlength=N)
    NBINS = NC * NBLK                     # 80 blocks
    cap = [125 + (1 if i < (N - 125 * NBINS) else 0) for i in range(NBINS)]
    heap = [(0, i) for i in range(NBINS)]
    heapq.heapify(heap)
    fill = [0] * NBINS
    bin_nodes: list[list[int]] = [[] for _ in range(NBINS)]
    for node in np.argsort(-deg, kind="stable"):
        spill = []
        while True:
            d, i = heapq.heappop(heap)
            if fill[i] < cap[i]:
                break
            spill.append((d, i))
        for s in spill:
            heapq.heappush(heap, s)
        bin_nodes[i].append(int(node))
        fill[i] += 1
        heapq.heappush(heap, (d + int(deg[node]), i))

    node_of_slot = np.full((NC, NPAD), -1, dtype=np.int64)
    for b in range(NBINS):
        c, blk = divmod(b, NBLK)
        nodes = bin_nodes[b]
        node_of_slot[c, blk * BLK:blk * BLK + len(nodes)] = nodes
    slot_core = np.empty(N, dtype=np.int64)
    slot_idx = np.empty(N, dtype=np.int64)
    for c in range(NC):
        real = node_of_slot[c] >= 0
        slot_core[node_of_slot[c][real]] = c
        slot_idx[node_of_slot[c][real]] = np.nonzero(real)[0]

    # ---- host prep: phase A inputs (permuted shards) ----
    import time
    _tm = {}
    _t = time.time()
    in_a = []
    for c in range(NC):
        nos = node_of_slot[c]
        real = nos >= 0
        kc = kernel[nos[nos >= 0]].astype(BF16)          # [NBLK*125, F, F]
        kt = np.ascontiguousarray(kc.transpose(1, 0, 2)).reshape(F, NBLK * 125 * F)
        xc = np.zeros((B, NPAD, F), dtype=BF16)
        xc[:, real] = x[:, nos[real]].astype(BF16)
        xt = np.ascontiguousarray(xc.transpose(2, 1, 0)).reshape(F, NPAD * B)
        in_a.append({"kt": kt, "xt": xt, "ident": ident})

    _tm["prep_a"] = time.time() - _t
    nc_a = _build_phase_a()
    _t = time.time()
    res_a = run_bass_kernel_spmd(nc_a, in_a, core_ids=list(range(NC)))
    _tm["run_a"] = time.time() - _t

    # full H in slot space: row (c*NPAD + slot) = h of node_of_slot[c, slot]
    Hfull = np.concatenate(
        [res_a.results[c]["hout"].reshape(NPAD, ROW) for c in range(NC)], axis=0)

    # ---- host: edge partition by src slot ----
    e_core = slot_core[edge_src]
    e_slot = slot_idx[edge_src]
    e_dstrow = (slot_core[edge_dst] * NPAD + slot_idx[edge_dst]).astype(np.int16)

    counts = np.zeros((NC, NBLK), dtype=np.int64)
    groups: list[list] = [[None] * NBLK for _ in range(NC)]
    for c in range(NC):
        m = e_core == c
        lc, dc, wc = e_slot[m], e_dstrow[m], edge_w[m]
        for blk in range(NBLK):
            mb = (lc // BLK) == blk
            # sort by dst row: ascending-address gather descriptors
            o2 = np.argsort(dc[mb], kind="stable")
            groups[c][blk] = ((lc[mb] % BLK)[o2], dc[mb][o2], wc[mb][o2])
            counts[c, blk] = int(mb.sum())

    nchunk = int(np.ceil(counts.max() / 128))
    nchunk += nchunk % 2          # quantize to even for compile-cache stability
    L = nchunk * 128

    colx = np.tile(np.arange(128, dtype=BF16), (128, 1))
    _t = time.time()
    in_b = []
    for c in range(NC):
        nos = node_of_slot[c]
        real = nos >= 0
        bc = np.zeros((NPAD, F), dtype=np.float32)
        bc[real] = bias[nos[real]]
        brow = np.tile(bc, (1, B)).reshape(NBLK, 128, ROW).astype(BF16)  # [n, b*128+g] rows

        owh = np.zeros((NBLK, 128, 2 * nchunk), dtype=np.float32)
        ixh = np.zeros((NBLK, 16, L // 16), dtype=np.int16)
        for blk in range(NBLK):
            off, dst, w = groups[c][blk]
            n = len(off)
            i = np.arange(n)
            owh[blk, i % 128, i // 128] = off
            owh[blk, i % 128, nchunk + i // 128] = w.astype(BF16)
            ixh[blk, i % 16, i // 16] = dst
        ixh = np.tile(ixh, (1, 8, 1))                   # replicate to 128 partitions
        # partition-contiguous layouts: [128, NBLK*...]
        brow2 = np.ascontiguousarray(brow.transpose(1, 0, 2)).reshape(128, NBLK * ROW)
        ix2 = np.ascontiguousarray(ixh.transpose(1, 0, 2)).reshape(128, NBLK * (L // 16))
        ow2 = np.ascontiguousarray(owh.transpose(1, 0, 2)).reshape(128, NBLK * 2 * nchunk)
        in_b.append({"hx": Hfull, "brow": brow2, "ix": ix2,
                     "ow": ow2, "colx": colx})

    _tm["prep_b"] = time.time() - _t
    nc_b = _build_phase_b(nchunk)
    _t = time.time()
    res_b = run_bass_kernel_spmd(nc_b, in_b, core_ids=list(range(NC)))
    _tm["run_b"] = time.time() - _t

    _t = time.time()
    out = np.empty((B, N, F), dtype=np.float32)
    for c in range(NC):
        nos = node_of_slot[c]
        real = nos >= 0
        o = res_b.results[c]["oout"][real].astype(np.float32).reshape(-1, B, F).transpose(1, 0, 2)
        out[:, nos[real]] = o
    _tm["assemble"] = time.time() - _t
    import sys as _sys
    print("kernel timing:", {k: round(v, 2) for k, v in _tm.items()}, file=_sys.stderr)
    return out
